# revision 1
# baseline (speedup 1.0000x reference)
"""T5-style encoder self-attention (dense_transformer) on 8 Trainium2 NeuronCores.

Problem (full shapes): hidden [2,2048,2048], Wq/Wk/Wv/Wo [2048,2048],
rel_emb [32,32] (bidirectional T5 relative-position bias), mask [2,1,1,2048].

Sharding: data-parallel over batch (2) x tensor-parallel over heads (4 groups
of 8 heads) = 8 cores, Megatron-style. Each core computes a partial output
[2048,2048] for its batch (its 8 heads through its Wo row-slice); the host
sums 4 partials per batch.

Per-core kernel design (bf16 operands everywhere, fp32 PSUM accumulation;
matmuls run at the full 1-cycle/row PE rate):
  - projections: Q^T,K^T [hd,s] layouts (hd on partitions) direct from
    lhsT=W, rhs=x^T; V [s,hd] from lhsT=x^T-slices, rhs=Wv. x^T supplied by
    the host (sharding-time layout prep).
  - Q^T is stored with s REVERSED so the relative-position bias becomes a
    positive-shear Toeplitz: U_h[p,j] = exp(bias_h)[diag = p+j-2047], built
    with one contiguous sheared DMA per head.
  - scores^T tiles [k=128part, q=512free]: row-packed pair of K=64 matmuls
    (tile_position (0,0)/(64,0)) computes 2 heads concurrently into the two
    banks of one [128,1024] PSUM tile; ONE ACT exp covers both heads.
  - softmax without max-subtraction (scores are O(1) by construction):
    ACT computes exp(s/8 + mask_k) psum->sbuf bf16; DVE multiplies by the
    Toeplitz exp-bias tile (bf16 2x mode).
  - PV with V_aug=[V | ones-block] (M=128): psum rows 64:128 replicate the
    softmax denominator for free; normalization is DEFERRED (denominator
    rows collected to DRAM, one compact reciprocal, broadcast back).
  - the kt loop is software-pipelined (QK emitted one iteration ahead) and
    head-pair p+1's Q/K projections are interleaved into pair p's attention
    so the PE never idles long enough for the HAM clock gate to throttle.
  - output projection: lhsT=ctx^T tiles, rhs=Wo rows, bf16, fp32 psum.

The relative-position bucket table is a host-side STRUCTURAL constant
(depends only on S, not on data); rel_emb values are gathered on device via a
one-hot matmul + exp.
"""

import math
import os
import sys

for _p in ("/opt/trn_rl_repo",):
    if _p not in sys.path:
        sys.path.insert(0, _p)

import numpy as np

import concourse.bass as bass
import concourse.mybir as mybir
import concourse.tile as tile
from concourse import bacc
from concourse.bass_utils import run_bass_kernel_spmd

DT = mybir.dt
AF = mybir.ActivationFunctionType
OP = mybir.AluOpType

# ---- problem constants (hardcoded per contract) ----
B, S, D = 2, 2048, 2048
N_HEADS, D_KV = 32, 64
NUM_BUCKETS, MAX_DISTANCE = 32, 128
NCORES = 8
HL = 8            # heads per core
P = 128
SC = 512          # free-dim chunk
NKT = S // P      # 16 k-tiles
NQC = S // SC     # 4 q-chunks
NDT = D // P      # 16 D-tiles
NMT = (HL * D_KV) // P   # 4 hd m-tiles per core
W_U = 3968        # toeplitz tile width: SC + (NKT-1)*P + ... = 512 + 1920*? -> k0+jg0 max 3456, +512
NDIAG = 4096      # ediag row stride (4095 used)


def _rel_bucket_host(d):
    """Exact numpy replica of reference._relative_position_bucket (fp32 math,
    int32 truncation) for bidirectional buckets. d = k - q (int array)."""
    num_buckets = NUM_BUCKETS // 2          # 16
    max_exact = num_buckets // 2            # 8
    rel = np.asarray(d, dtype=np.int64)
    buckets = (rel > 0).astype(np.int32) * num_buckets
    arel = np.abs(rel)
    is_small = arel < max_exact
    rp_safe = np.maximum(arel, 1).astype(np.float32)
    log_ratio = np.log(rp_safe / np.float32(max_exact)).astype(np.float32)
    scale = np.float32(math.log(MAX_DISTANCE / max_exact))
    rp_large = max_exact + (log_ratio / scale * np.float32(num_buckets - max_exact)).astype(np.int32)
    rp_large = np.minimum(rp_large, num_buckets - 1)
    buckets = buckets + np.where(is_small, arel.astype(np.int32), rp_large)
    return buckets.astype(np.int32)


def _onehot_const():
    """OH[u, i] = 1 if bucket(i - 2047) == u, i in [0, 4095); col 4095 = 0."""
    i = np.arange(NDIAG - 1)
    b = _rel_bucket_host(i - (S - 1))
    oh = np.zeros((NUM_BUCKETS, NDIAG), dtype=np.float32)
    oh[b, i] = 1.0
    return oh


def _build():
    nc = bacc.Bacc(None, name="attn_tp")

    xt = nc.declare_dram_parameter("xt", [D, S], DT.bfloat16, isOutput=False)
    wq = nc.declare_dram_parameter("wq", [D, HL * D_KV], DT.bfloat16, isOutput=False)
    wk = nc.declare_dram_parameter("wk", [D, HL * D_KV], DT.bfloat16, isOutput=False)
    wv = nc.declare_dram_parameter("wv", [D, HL * D_KV], DT.bfloat16, isOutput=False)
    wo = nc.declare_dram_parameter("wo", [HL * D_KV, D], DT.bfloat16, isOutput=False)
    mask = nc.declare_dram_parameter("mask", [S], DT.float32, isOutput=False)
    rel = nc.declare_dram_parameter("rel", [NUM_BUCKETS, HL], DT.float32, isOutput=False)
    oh = nc.declare_dram_parameter("oh", [NUM_BUCKETS, NDIAG], DT.float32, isOutput=False)
    out = nc.declare_dram_parameter("out", [S, D], DT.float32, isOutput=True)

    with tile.TileContext(nc) as tc:
        with (
            tc.tile_pool(name="res", bufs=1) as res,          # persistent tensors
            tc.tile_pool(name="xtp", bufs=3) as xtp,          # x^T stream tiles
            tc.tile_pool(name="stage", bufs=2) as stage,      # fp32 staging
            tc.tile_pool(name="upool", bufs=3) as upool,      # toeplitz exp-bias tiles
            tc.tile_pool(name="pexp", bufs=5) as pexpp,       # probs tiles
            tc.tile_pool(name="outp", bufs=2) as outp,        # out staging
            tc.tile_pool(name="psum", bufs=4, space="PSUM") as psum,  # [128,1024] slots
            tc.tile_pool(name="dram", bufs=1, space="DRAM") as dramp,
        ):
            # ---------- phase 0: constants / ediag ----------
            mask_sb = res.tile([P, NKT], DT.float32, tag="mask")
            nc.sync.dma_start(mask_sb[:], mask.ap().rearrange("(kt p) -> p kt", p=P))

            rel_sb = res.tile([NUM_BUCKETS, HL], DT.float32, tag="rel")
            nc.sync.dma_start(rel_sb[:], rel[:])

            ediag_dram = dramp.tile([HL, NDIAG], DT.bfloat16)
            den_dram = dramp.tile([HL * NQC, SC], DT.float32)
            rcp_dram = dramp.tile([HL * NQC, SC], DT.float32)
            for c in range(NDIAG // SC):
                oh_sb = stage.tile([NUM_BUCKETS, SC], DT.float32, tag="oh")
                nc.sync.dma_start(oh_sb[:], oh[:, c * SC:(c + 1) * SC])
                ed_ps = psum.tile([P, 2 * SC], DT.float32, tag="ps", name=f"edps{c}")[:HL, 0:SC]
                nc.tensor.matmul(ed_ps[:], rel_sb[:], oh_sb[:], start=True, stop=True)
                ed_sb = stage.tile([HL, SC], DT.bfloat16, tag="ed_sb")
                nc.scalar.activation(out=ed_sb[:], in_=ed_ps[:], func=AF.Exp)
                nc.sync.dma_start(ediag_dram[:, c * SC:(c + 1) * SC], ed_sb[:])

            # weights (resident, bf16)
            wq_sb = res.tile([P, NDT, HL * D_KV], DT.bfloat16, tag="wq")
            wk_sb = res.tile([P, NDT, HL * D_KV], DT.bfloat16, tag="wk")
            wv_sb = res.tile([P, NDT, HL * D_KV], DT.bfloat16, tag="wv")
            nc.sync.dma_start(wq_sb[:], wq.ap().rearrange("(kt p) h -> p kt h", p=P))
            nc.sync.dma_start(wk_sb[:], wk.ap().rearrange("(kt p) h -> p kt h", p=P))
            nc.sync.dma_start(wv_sb[:], wv.ap().rearrange("(kt p) h -> p kt h", p=P))
            wo_sb = res.tile([P, NMT, D], DT.bfloat16, tag="wo")
            nc.sync.dma_start(wo_sb[:], wo.ap().rearrange("(mt p) d -> p mt d", p=P))

            # persistent activations
            qt_sb = res.tile([P, NMT, S], DT.bfloat16, tag="qt")   # q REVERSED
            kt_sb = res.tile([P, NMT, S], DT.bfloat16, tag="kt")
            vaug = res.tile([P, NKT, HL, 2 * D_KV], DT.bfloat16, tag="vaug")
            ctxt = res.tile([P, NMT, S], DT.bfloat16, tag="ctxt")
            nc.vector.memset(vaug[:], 1.0)

            def proj_qk_chunk(pr, nq):
                """Q^T/K^T m-tile pr, s-chunk nq (pair pr's heads)."""
                qk_ps = psum.tile([P, 2 * SC], DT.float32, tag="ps",
                                  name=f"qkps{pr}_{nq}")
                q_ps, k_ps = qk_ps[:, 0:SC], qk_ps[:, SC:2 * SC]
                for kd in range(NDT):
                    xt_t = xtp.tile([P, SC], DT.bfloat16, tag="xt",
                                    name=f"xq{pr}_{nq}_{kd}")
                    nc.sync.dma_start(
                        xt_t[:], xt[kd * P:(kd + 1) * P, nq * SC:(nq + 1) * SC]
                    )
                    nc.tensor.matmul(
                        q_ps, wq_sb[:, kd, pr * P:(pr + 1) * P], xt_t[:],
                        start=(kd == 0), stop=(kd == NDT - 1),
                    )
                    nc.tensor.matmul(
                        k_ps, wk_sb[:, kd, pr * P:(pr + 1) * P], xt_t[:],
                        start=(kd == 0), stop=(kd == NDT - 1),
                    )
                dst = qt_sb[:, pr, :]
                rev = bass.AP(
                    tensor=dst.tensor,
                    offset=dst.offset + (S - 1 - nq * SC),
                    ap=[list(dst.ap[0]), [-1, SC]],
                )
                nc.vector.tensor_copy(rev, q_ps)
                nc.vector.tensor_copy(kt_sb[:, pr, nq * SC:(nq + 1) * SC], k_ps)

            def proj_v_chunk(nq):
                """V (all heads), s-chunk nq -> vaug[:, :, :, 0:64]."""
                v_pair = [psum.tile([P, 2 * SC], DT.float32, tag="ps",
                                    name=f"vps{nq}_{i}") for i in range(2)]
                v_ps = [v_pair[0][:, 0:SC], v_pair[0][:, SC:2 * SC],
                        v_pair[1][:, 0:SC], v_pair[1][:, SC:2 * SC]]
                for kd in range(NDT):
                    xt_t = xtp.tile([P, SC], DT.bfloat16, tag="xt",
                                    name=f"xv{nq}_{kd}")
                    nc.sync.dma_start(
                        xt_t[:], xt[kd * P:(kd + 1) * P, nq * SC:(nq + 1) * SC]
                    )
                    for st in range(4):
                        nc.tensor.matmul(
                            v_ps[st], xt_t[:, st * P:(st + 1) * P],
                            wv_sb[:, kd, :],
                            start=(kd == 0), stop=(kd == NDT - 1),
                        )
                for st in range(4):
                    kt_glob = nq * 4 + st
                    nc.vector.tensor_copy(
                        vaug[:, kt_glob, :, 0:D_KV],
                        v_ps[st].rearrange("p (h d) -> p h d", d=D_KV),
                    )

            def attn_qc(pr, qc, u_t):
                """Attention for head pair pr, reversed-q chunk qc.
                kt loop is software-pipelined: QK(kt+1) is emitted before
                PV(kt) so the in-order PE queue never waits on exp/mult."""
                h0, h1 = 2 * pr, 2 * pr + 1
                jg0 = qc * SC
                cx01 = psum.tile([P, 2 * SC], DT.float32, tag="ps",
                                 name=f"cx{pr}_{qc}")
                cx0, cx1 = cx01[:, 0:SC], cx01[:, SC:2 * SC]

                def emit_qk(kt):
                    s01 = psum.tile([P, 2 * SC], DT.float32, tag="ps",
                                    name=f"s{pr}_{qc}_{kt}")
                    nc.tensor.matmul(
                        s01[:, 0:SC], kt_sb[0:64, pr, kt * P:(kt + 1) * P],
                        qt_sb[0:64, pr, jg0:jg0 + SC],
                        start=True, stop=True, tile_position=(0, 0),
                    )
                    nc.tensor.matmul(
                        s01[:, SC:2 * SC], kt_sb[64:128, pr, kt * P:(kt + 1) * P],
                        qt_sb[64:128, pr, jg0:jg0 + SC],
                        start=True, stop=True, tile_position=(64, 0),
                    )
                    return s01

                s01 = emit_qk(0)
                for kt in range(NKT):
                    s01_next = emit_qk(kt + 1) if kt + 1 < NKT else None
                    px = pexpp.tile([P, 2 * SC], DT.bfloat16, tag="pexp",
                                    name=f"px{pr}_{qc}_{kt}")
                    nc.scalar.activation(
                        out=px[:], in_=s01[:], func=AF.Exp,
                        bias=mask_sb[:, kt:kt + 1], scale=1.0 / math.sqrt(D_KV),
                    )
                    j0 = kt * P + jg0
                    nc.vector.tensor_tensor(
                        px[:, 0:SC], px[:, 0:SC], u_t[h0][:, j0:j0 + SC], OP.mult
                    )
                    nc.vector.tensor_tensor(
                        px[:, SC:2 * SC], px[:, SC:2 * SC],
                        u_t[h1][:, j0:j0 + SC], OP.mult
                    )
                    nc.tensor.matmul(
                        cx0, vaug[:, kt, h0, :], px[:, 0:SC],
                        start=(kt == 0), stop=(kt == NKT - 1),
                    )
                    nc.tensor.matmul(
                        cx1, vaug[:, kt, h1, :], px[:, SC:2 * SC],
                        start=(kt == 0), stop=(kt == NKT - 1),
                    )
                    s01 = s01_next
                for hh, cx in ((h0, cx0), (h1, cx1)):
                    # unnormalized ctx (un-reversing q) + stash denominator row
                    base = ctxt[(hh % 2) * 64:(hh % 2) * 64 + 64, pr, :]
                    dst = bass.AP(
                        tensor=base.tensor,
                        offset=base.offset + (S - 1 - jg0),
                        ap=[list(base.ap[0]), [-1, SC]],
                    )
                    nc.scalar.copy(dst, cx[0:D_KV, :])
                    dn = stage.tile([P, SC], DT.float32, tag="dn",
                                    name=f"dn{hh}_{qc}")
                    dslc = dn[64:65, :]
                    drev = bass.AP(
                        tensor=dslc.tensor,
                        offset=dslc.offset + (SC - 1),
                        ap=[list(dslc.ap[0]), [-1, SC]],
                    )
                    nc.vector.tensor_copy(drev, cx[64:65, :])
                    nc.sync.dma_start(den_dram[hh * NQC + qc, :], dn[64:65, :])

            def load_u(pr):
                u_t = {}
                for hh in (2 * pr, 2 * pr + 1):
                    u = upool.tile([P, W_U], DT.bfloat16, tag="u", name=f"u{hh}")
                    shear = bass.AP(
                        tensor=ediag_dram.tensor,
                        offset=ediag_dram.offset + hh * NDIAG,
                        ap=[[1, P], [1, W_U]],
                    )
                    nc.sync.dma_start(u[:], shear)
                    u_t[hh] = u
                return u_t

            def normalize_qc(pr, qc):
                """Per-qc normalization (used for the last pair to avoid a
                serial tail before the output projection)."""
                den_sb = stage.tile([2, SC], DT.float32, tag="den8",
                                    name=f"dq{pr}_{qc}")
                rows = [2 * pr * NQC + qc, (2 * pr + 1) * NQC + qc]
                for r, row in enumerate(rows):
                    nc.sync.dma_start(den_sb[r:r + 1, :], den_dram[row, :])
                rcp2 = stage.tile([2, SC], DT.float32, tag="rcp8",
                                  name=f"rq{pr}_{qc}")
                nc.vector.reciprocal(rcp2[:], den_sb[:])
                for r, row in enumerate(rows):
                    nc.sync.dma_start(rcp_dram[row, :], rcp2[r:r + 1, :])
                for r in range(2):
                    hh = 2 * pr + r
                    idx = hh * NQC + qc
                    off = (hh % 2) * 64
                    rb = stage.tile([P, SC], DT.float32, tag="rb",
                                    name=f"rbq{hh}_{qc}")
                    bcast = bass.AP(
                        tensor=rcp_dram.tensor,
                        offset=rcp_dram.offset + idx * SC,
                        ap=[[0, D_KV], [1, SC]],
                    )
                    nc.sync.dma_start(rb[off:off + D_KV, :], bcast)
                    q0t = S - (qc + 1) * SC
                    cslc = ctxt[off:off + 64, hh // 2, q0t:q0t + SC]
                    nc.vector.tensor_tensor(
                        cslc, cslc, rb[off:off + D_KV, :], OP.mult
                    )

            def normalize_pair(pr):
                """Deferred softmax division for pair pr's rows of ctxt."""
                den_sb = stage.tile([2 * NQC, SC], DT.float32, tag="den8",
                                    name=f"den{pr}")
                nc.sync.dma_start(den_sb[:], den_dram[2 * pr * NQC:(2 * pr + 2) * NQC, :])
                rcp8 = stage.tile([2 * NQC, SC], DT.float32, tag="rcp8",
                                  name=f"rcp{pr}")
                nc.vector.reciprocal(rcp8[:], den_sb[:])
                nc.sync.dma_start(rcp_dram[2 * pr * NQC:(2 * pr + 2) * NQC, :], rcp8[:])
                for hh in (2 * pr, 2 * pr + 1):
                    for qc in range(NQC):
                        idx = hh * NQC + qc
                        off = (hh % 2) * 64
                        rb = stage.tile([P, SC], DT.float32, tag="rb",
                                        name=f"rb{hh}_{qc}")
                        bcast = bass.AP(
                            tensor=rcp_dram.tensor,
                            offset=rcp_dram.offset + idx * SC,
                            ap=[[0, D_KV], [1, SC]],
                        )
                        nc.sync.dma_start(rb[off:off + D_KV, :], bcast)
                        q0t = S - (qc + 1) * SC
                        cslc = ctxt[off:off + 64, hh // 2, q0t:q0t + SC]
                        nc.vector.tensor_tensor(
                            cslc, cslc, rb[off:off + D_KV, :], OP.mult
                        )

            # ---------- phase 1: pair-0 Q/K, then V (all heads) ----------
            for nq in range(NQC):
                proj_qk_chunk(0, nq)
            for nq in range(NQC):
                proj_v_chunk(nq)

            # ---------- phase 2: attention pipelined with next pair's Q/K ----
            u_t = load_u(0)
            last = HL // 2 - 1
            for pr in range(HL // 2):
                if pr + 1 <= last:
                    next_u = load_u(pr + 1)
                for qc in range(NQC):
                    attn_qc(pr, qc, u_t)
                    if pr + 1 <= last:
                        proj_qk_chunk(pr + 1, qc)
                normalize_pair(pr)
                if pr + 1 <= last:
                    u_t = next_u

            # ---------- phase 3: output projection ----------
            for st in range(NKT):
                for nd in range(NQC):
                    o_ps = psum.tile([P, 2 * SC], DT.float32, tag="ps",
                                     name=f"ops{st}_{nd}")[:, 0:SC]
                    for m in range(NMT):
                        nc.tensor.matmul(
                            o_ps, ctxt[:, m, st * P:(st + 1) * P],
                            wo_sb[:, m, nd * SC:(nd + 1) * SC],
                            start=(m == 0), stop=(m == NMT - 1),
                        )
                    o_t = outp.tile([P, SC], DT.float32, tag="out",
                                    name=f"ot{st}_{nd}")
                    nc.scalar.copy(o_t[:], o_ps)
                    nc.sync.dma_start(
                        out[st * P:(st + 1) * P, nd * SC:(nd + 1) * SC], o_t[:]
                    )

    nc.finalize()
    return nc


_NC_CACHE = None


def _get_nc():
    global _NC_CACHE
    if _NC_CACHE is None:
        _NC_CACHE = _build()
    return _NC_CACHE


def _in_maps(hidden_states, attention_mask, Wq, Wk, Wv, Wo, rel_emb):
    oh = _onehot_const()
    import ml_dtypes
    bf16 = ml_dtypes.bfloat16
    maps = []
    for c in range(NCORES):
        b, g = c // 4, c % 4
        hlo, hhi = g * HL, (g + 1) * HL
        maps.append({
            "xt": np.ascontiguousarray(hidden_states[b].T).astype(bf16),
            "wq": np.ascontiguousarray(Wq[:, hlo * D_KV:hhi * D_KV]).astype(bf16),
            "wk": np.ascontiguousarray(Wk[:, hlo * D_KV:hhi * D_KV]).astype(bf16),
            "wv": np.ascontiguousarray(Wv[:, hlo * D_KV:hhi * D_KV]).astype(bf16),
            "wo": np.ascontiguousarray(Wo[hlo * D_KV:hhi * D_KV, :]).astype(bf16),
            "mask": np.ascontiguousarray(attention_mask[b, 0, 0, :]).astype(np.float32),
            "rel": np.ascontiguousarray(rel_emb[:, hlo:hhi]).astype(np.float32),
            "oh": oh,
        })
    return maps


def kernel(hidden_states, attention_mask, Wq, Wk, Wv, Wo, rel_emb, _trace=False,
           _trace_kwargs=None):
    hidden_states = np.asarray(hidden_states, dtype=np.float32)
    attention_mask = np.asarray(attention_mask, dtype=np.float32)
    Wq = np.asarray(Wq, dtype=np.float32)
    Wk = np.asarray(Wk, dtype=np.float32)
    Wv = np.asarray(Wv, dtype=np.float32)
    Wo = np.asarray(Wo, dtype=np.float32)
    rel_emb = np.asarray(rel_emb, dtype=np.float32)

    nc = _get_nc()
    maps = _in_maps(hidden_states, attention_mask, Wq, Wk, Wv, Wo, rel_emb)
    kw = dict(_trace_kwargs or {})
    res = run_bass_kernel_spmd(nc, maps, core_ids=list(range(NCORES)),
                               trace=_trace, **kw)
    kernel.last_results = res
    outp = np.empty((B, S, D), dtype=np.float32)
    for b in range(B):
        acc = np.asarray(res.results[4 * b]["out"], dtype=np.float32).copy()
        for g in range(1, 4):
            acc += np.asarray(res.results[4 * b + g]["out"], dtype=np.float32)
        outp[b] = acc
    return outp



# revision 9
# speedup vs baseline: 1.0976x; 1.0976x over previous
"""T5-style encoder self-attention (dense_transformer) on 8 Trainium2 NeuronCores.

Problem (full shapes): hidden [2,2048,2048], Wq/Wk/Wv/Wo [2048,2048],
rel_emb [32,32] (bidirectional T5 relative-position bias), mask [2,1,1,2048].

Sharding: data-parallel over batch (2) x tensor-parallel over heads (4 groups
of 8 heads) = 8 cores, Megatron-style. Each core computes a partial output
[2048,2048] for its batch (its 8 heads through its Wo row-slice); the host
sums 4 partials per batch.

Per-core kernel design (bf16 operands, fp32 PSUM accumulation):
  - Both relative-position bias diagonal tables are HOST-computed (they are
    data-dependent only through rel_emb, a [32,32] input): brel = 8*bias
    (log domain, for additive injection) and erel = exp(bias) (for
    multiplicative application), each [8 heads, 4096 diagonals] bf16.
  - Phase B: single pass over x^T computes pair-0 Q^T/K^T and V for ALL
    heads (6 matmuls per x^T tile, PE-bound).  Q^T is stored with s
    REVERSED so the bias becomes a positive-shear Toeplitz.
  - Phase C attention, per (head-pair, q-chunk), k-tile loop pipelined one
    iteration ahead:
      * near-diagonal k-tiles (|k-q| < ~91 somewhere in tile): the bias tile
        is INJECTED into PSUM via an identity matmul (start=True), then the
        two packed QK matmuls accumulate on top; ACT computes
        exp(s/8 + mask + bias) in one shot - no DVE multiply.
      * far k-tiles: bias is exactly constant (bucket 15/31 saturates), but
        rather than bake runtime constants we keep the baseline DVE multiply
        with the erel shear tile (DVE is otherwise idle).
      * next-pair Q/K projection matmuls are interleaved PER k-tile so they
        fill the PE's ACT-wait bubbles (the in-order PE queue can only run
        work that is already emitted ahead of the blocked PV matmul).
  - V augmentation: per pair, even head block = [v(0:64) | ones(64)] (M=65,
    denominator lands on PSUM partition 64), odd head block = 128 wide with
    ones at col 32 and v at cols 64:128 (denominator on partition 32, ctx on
    partitions 64:128).  This keeps every subsequent normalize op
    partition-aligned: reciprocal_approx_fast on the denominator rows, a K=1
    ones-matmul broadcasts the reciprocal across partitions, and a single
    fused DVE tensor_tensor does normalize + un-reverse + bf16 writeback.
    No DRAM round trips.
  - Phase D output projection: for each s-tile, loop m inside nd so
    consecutive matmuls target different PSUM banks; evacuation alternates
    ACT/DVE.
"""

import math
import sys

for _p in ("/opt/trn_rl_repo",):
    if _p not in sys.path:
        sys.path.insert(0, _p)

import numpy as np

import concourse.bass as bass
import concourse.mybir as mybir
import concourse.tile as tile
from concourse import bacc
from concourse.bass_utils import run_bass_kernel_spmd

DT = mybir.dt
AF = mybir.ActivationFunctionType
OP = mybir.AluOpType

# ---- problem constants (hardcoded per contract) ----
B, S, D = 2, 2048, 2048
N_HEADS, D_KV = 32, 64
NUM_BUCKETS, MAX_DISTANCE = 32, 128
NCORES = 8
HL = 8            # heads per core
P = 128
SC = 512          # free-dim chunk
NKT = S // P      # 16 k-tiles
NQC = S // SC     # 4 q-chunks
NDT = D // P      # 16 D-tiles
NMT = (HL * D_KV) // P   # 4 hd m-tiles per core
NPAIR = HL // 2   # 4 head pairs per core
W_U = 3968        # full exp-table shear tile width
W_NEAR = 1152     # near-window raw-table shear tile width
NDIAG = 4096
VW = 193          # vaug per-(kt,pair) width: even block 65 + odd block 128

# near-tile bookkeeping: tile (kt, qc) is "far" iff |k-q| >= 91 everywhere
def _is_near(kt, qc):
    dmin = 128 * kt - 512 * qc - 511
    dmax = 128 * kt + 127 - 512 * qc
    return not (dmin >= 91 or dmax <= -91)

NEAR = {(kt, qc): _is_near(kt, qc) for kt in range(NKT) for qc in range(NQC)}
# raw-table window base per qc (clamped so the shear read stays in bounds)
B0 = [max(0, 1024 * qc - 128) for qc in range(NQC)]
WQC = [min(W_NEAR, NDIAG - 127 - b0) for b0 in B0]


def _rel_bucket_host(d):
    """Exact numpy replica of reference._relative_position_bucket."""
    num_buckets = NUM_BUCKETS // 2          # 16
    max_exact = num_buckets // 2            # 8
    rel = np.asarray(d, dtype=np.int64)
    buckets = (rel > 0).astype(np.int32) * num_buckets
    arel = np.abs(rel)
    is_small = arel < max_exact
    rp_safe = np.maximum(arel, 1).astype(np.float32)
    log_ratio = np.log(rp_safe / np.float32(max_exact)).astype(np.float32)
    scale = np.float32(math.log(MAX_DISTANCE / max_exact))
    rp_large = max_exact + (log_ratio / scale * np.float32(num_buckets - max_exact)).astype(np.int32)
    rp_large = np.minimum(rp_large, num_buckets - 1)
    buckets = buckets + np.where(is_small, arel.astype(np.int32), rp_large)
    return buckets.astype(np.int32)


def _bias_tables(rel_emb_slice):
    """rel_emb_slice: [NUM_BUCKETS, HL] fp32 -> (brel, erel) [HL, NDIAG].
    brel[h, i] = 8 * bias(d = i - 2047); erel[h, i] = exp(bias)."""
    import ml_dtypes
    i = np.arange(NDIAG - 1)
    b = _rel_bucket_host(i - (S - 1))                  # [4095]
    vals = rel_emb_slice[b, :]                         # [4095, HL] fp32
    brel = np.zeros((HL, NDIAG), dtype=np.float32)
    erel = np.zeros((HL, NDIAG), dtype=np.float32)
    brel[:, : NDIAG - 1] = 8.0 * vals.T
    erel[:, : NDIAG - 1] = np.exp(vals.T)
    return (brel.astype(ml_dtypes.bfloat16), erel.astype(ml_dtypes.bfloat16))


def _build():
    nc = bacc.Bacc(None, name="attn_tp")

    xt = nc.declare_dram_parameter("xt", [D, S], DT.bfloat16, isOutput=False)
    wq = nc.declare_dram_parameter("wq", [D, HL * D_KV], DT.bfloat16, isOutput=False)
    wk = nc.declare_dram_parameter("wk", [D, HL * D_KV], DT.bfloat16, isOutput=False)
    wv = nc.declare_dram_parameter("wv", [D, HL * D_KV], DT.bfloat16, isOutput=False)
    wo = nc.declare_dram_parameter("wo", [HL * D_KV, D], DT.bfloat16, isOutput=False)
    mask = nc.declare_dram_parameter("mask", [S], DT.float32, isOutput=False)
    brel = nc.declare_dram_parameter("brel", [HL, NDIAG], DT.bfloat16, isOutput=False)
    erel = nc.declare_dram_parameter("erel", [HL, NDIAG], DT.bfloat16, isOutput=False)
    ident = nc.declare_dram_parameter("ident", [P, P], DT.bfloat16, isOutput=False)
    out = nc.declare_dram_parameter("out", [S, D], DT.float32, isOutput=True)

    with tile.TileContext(nc) as tc:
        with (
            tc.tile_pool(name="res", bufs=1) as res,          # persistent tensors
            tc.tile_pool(name="xtp", bufs=3) as xtp,          # x^T stream tiles
            tc.tile_pool(name="upool", bufs=3) as upool,      # exp-bias shear tiles
            tc.tile_pool(name="urawp", bufs=2) as urawp,      # raw-bias near windows
            tc.tile_pool(name="pexp", bufs=3) as pexpp,       # probs tiles
            tc.tile_pool(name="stage", bufs=2) as stage,      # normalize staging
            tc.tile_pool(name="outp", bufs=3) as outp,        # out staging
            tc.tile_pool(name="psum", bufs=4, space="PSUM") as psum,  # [128,1024] slots
        ):
            # ---------- constants ----------
            mask_sb = res.tile([P, NKT], DT.float32, tag="mask")
            nc.sync.dma_start(mask_sb[:], mask.ap().rearrange("(kt p) -> p kt", p=P))

            id_sb = res.tile([P, P], DT.bfloat16, tag="ident")
            nc.sync.dma_start(id_sb[:], ident[:])

            onesb = res.tile([P, P], DT.bfloat16, tag="onesb")
            nc.vector.memset(onesb[:], 1.0)

            # weights (resident, bf16)
            wq_sb = res.tile([P, NDT, HL * D_KV], DT.bfloat16, tag="wq")
            wk_sb = res.tile([P, NDT, HL * D_KV], DT.bfloat16, tag="wk")
            wv_sb = res.tile([P, NDT, HL * D_KV], DT.bfloat16, tag="wv")
            nc.sync.dma_start(wq_sb[:], wq.ap().rearrange("(kt p) h -> p kt h", p=P))
            nc.sync.dma_start(wk_sb[:], wk.ap().rearrange("(kt p) h -> p kt h", p=P))
            nc.sync.dma_start(wv_sb[:], wv.ap().rearrange("(kt p) h -> p kt h", p=P))
            wo_sb = res.tile([P, NMT, D], DT.bfloat16, tag="wo")
            nc.sync.dma_start(wo_sb[:], wo.ap().rearrange("(mt p) d -> p mt d", p=P))

            # persistent activations
            qt_sb = res.tile([P, NMT, S], DT.bfloat16, tag="qt")   # q REVERSED
            kt_sb = res.tile([P, NMT, S], DT.bfloat16, tag="kt")
            vaug = res.tile([P, NKT, NPAIR, VW], DT.bfloat16, tag="vaug")
            ctxt = res.tile([P, NMT, S], DT.bfloat16, tag="ctxt")
            nc.vector.memset(vaug[:], 1.0)
            rb16 = res.tile([P, SC], DT.bfloat16, tag="rb16")
            nc.vector.memset(rb16[:], 0.0)

            # ACT exp table warm-up (hide the ~2.7us table load under phase B)
            warm = res.tile([1, 2], DT.float32, tag="warm")
            nc.scalar.activation(out=warm[0:1, 0:1], in_=mask_sb[0:1, 0:1], func=AF.Exp)

            def rev_ap(base, jg0):
                """reversed-q view: base is a [rows, S] AP slice of a res
                tensor; returns [rows, SC] AP walking q backwards so writing
                reversed data lands in natural order."""
                return bass.AP(
                    tensor=base.tensor,
                    offset=base.offset + (S - 1 - jg0),
                    ap=[list(base.ap[0]), [-1, SC]],
                )

            def load_u(pr):
                """full exp-table shear tiles for pair pr's two heads."""
                u_t = {}
                for hh in (2 * pr, 2 * pr + 1):
                    u = upool.tile([P, W_U], DT.bfloat16, tag="u", name=f"u{hh}", bufs=4)
                    ap0 = erel.ap()
                    shear = bass.AP(
                        tensor=ap0.tensor,
                        offset=ap0.offset + hh * NDIAG,
                        ap=[[1, P], [1, W_U]],
                    )
                    nc.sync.dma_start(u[:], shear)
                    u_t[hh] = u
                return u_t

            def load_uraw(pr, qc):
                """near-window raw-bias shear tile [P, 2, W_NEAR] for (pr, qc)."""
                w = WQC[qc]
                t = urawp.tile([P, 2, W_NEAR], DT.bfloat16, tag="uraw",
                               name=f"uraw{pr}_{qc}")
                ap0 = brel.ap()
                for i, hh in enumerate((2 * pr, 2 * pr + 1)):
                    shear = bass.AP(
                        tensor=ap0.tensor,
                        offset=ap0.offset + hh * NDIAG + B0[qc],
                        ap=[[1, P], [1, w]],
                    )
                    nc.sync.dma_start(t[:, i, 0:w], shear)
                return t

            # ---------- phase B: pair-0 Q/K + V (all heads), single x^T pass ----
            for nq in range(NQC):
                qk_ps = psum.tile([P, 2 * SC], DT.float32, tag="ps",
                                  name=f"qkps0_{nq}")
                q_ps, k_ps = qk_ps[:, 0:SC], qk_ps[:, SC:2 * SC]
                v01 = psum.tile([P, 2 * SC], DT.float32, tag="ps", name=f"v01_{nq}")
                v23 = psum.tile([P, 2 * SC], DT.float32, tag="ps", name=f"v23_{nq}")
                v_ps = [v01[:, 0:SC], v01[:, SC:2 * SC],
                        v23[:, 0:SC], v23[:, SC:2 * SC]]
                for kd in range(NDT):
                    xt_t = xtp.tile([P, SC], DT.bfloat16, tag="xt",
                                    name=f"xb{nq}_{kd}")
                    nc.sync.dma_start(
                        xt_t[:], xt[kd * P:(kd + 1) * P, nq * SC:(nq + 1) * SC]
                    )
                    nc.tensor.matmul(
                        q_ps, wq_sb[:, kd, 0:P], xt_t[:],
                        start=(kd == 0), stop=(kd == NDT - 1),
                    )
                    nc.tensor.matmul(
                        k_ps, wk_sb[:, kd, 0:P], xt_t[:],
                        start=(kd == 0), stop=(kd == NDT - 1),
                    )
                    for st in range(4):
                        nc.tensor.matmul(
                            v_ps[st], xt_t[:, st * P:(st + 1) * P],
                            wv_sb[:, kd, :],
                            start=(kd == 0), stop=(kd == NDT - 1),
                        )
                # drain: q (reversed) + k casts, V -> vaug blocks
                nc.vector.tensor_copy(rev_ap(qt_sb[:, 0, :], nq * SC), q_ps)
                nc.vector.tensor_copy(kt_sb[:, 0, nq * SC:(nq + 1) * SC], k_ps)
                for st in range(4):
                    ktg = nq * 4 + st
                    vsrc = v_ps[st].rearrange("p (pr par d) -> p pr par d",
                                              par=2, d=D_KV)
                    nc.vector.tensor_copy(vaug[:, ktg, :, 0:D_KV],
                                          vsrc[:, :, 0, :])
                    nc.vector.tensor_copy(vaug[:, ktg, :, 129:193],
                                          vsrc[:, :, 1, :])

            # ---------- phase C: attention, proj of pair pr+1 interleaved ----
            def attn_qc(pr, qc, u_t, uraw_t, proj):
                """attention for head pair pr, reversed-q chunk qc.
                proj: None or pr+1 (emit that pair's Q/K proj, 1 kd per kt).
                Emission order per kt puts all independent PE work BEFORE the
                dependent PV matmuls so the in-order PE queue can fill
                ACT-wait bubbles."""
                h0, h1 = 2 * pr, 2 * pr + 1
                jg0 = qc * SC
                cx01 = psum.tile([P, 2 * SC], DT.float32, tag="ps",
                                 name=f"cx{pr}_{qc}")
                if proj is not None:
                    pj_ps = psum.tile([P, 2 * SC], DT.float32, tag="ps",
                                      name=f"pjps{proj}_{qc}")
                    pjq, pjk = pj_ps[:, 0:SC], pj_ps[:, SC:2 * SC]

                def emit_s(kt):
                    s01 = psum.tile([P, 2 * SC], DT.float32, tag="ps",
                                    name=f"s{pr}_{qc}_{kt}")
                    near = NEAR[(kt, qc)]
                    j0 = kt * P + jg0
                    if near:
                        a = j0 - B0[qc]
                        nc.tensor.matmul(
                            s01[:, 0:SC], id_sb[:], uraw_t[:, 0, a:a + SC],
                            start=True, stop=False,
                        )
                        nc.tensor.matmul(
                            s01[:, SC:2 * SC], id_sb[:], uraw_t[:, 1, a:a + SC],
                            start=True, stop=False,
                        )
                    nc.tensor.matmul(
                        s01[:, 0:SC], kt_sb[0:64, pr, kt * P:(kt + 1) * P],
                        qt_sb[0:64, pr, jg0:jg0 + SC],
                        start=not near, stop=True, tile_position=(0, 0),
                    )
                    nc.tensor.matmul(
                        s01[:, SC:2 * SC], kt_sb[64:128, pr, kt * P:(kt + 1) * P],
                        qt_sb[64:128, pr, jg0:jg0 + SC],
                        start=not near, stop=True, tile_position=(64, 0),
                    )
                    return s01

                def emit_proj(kd):
                    xt_t = xtp.tile([P, SC], DT.bfloat16, tag="xt",
                                    name=f"xp{proj}_{qc}_{kd}")
                    nc.sync.dma_start(
                        xt_t[:], xt[kd * P:(kd + 1) * P, jg0:jg0 + SC]
                    )
                    nc.tensor.matmul(
                        pjq, wq_sb[:, kd, proj * P:(proj + 1) * P], xt_t[:],
                        start=(kd == 0), stop=(kd == NDT - 1),
                    )
                    nc.tensor.matmul(
                        pjk, wk_sb[:, kd, proj * P:(proj + 1) * P], xt_t[:],
                        start=(kd == 0), stop=(kd == NDT - 1),
                    )

                s01 = emit_s(0)
                for kt in range(NKT):
                    s01_next = emit_s(kt + 1) if kt + 1 < NKT else None
                    if proj is not None:
                        emit_proj(kt)
                    px = pexpp.tile([P, 2 * SC], DT.bfloat16, tag="pexp",
                                    name=f"px{pr}_{qc}_{kt}")
                    nc.scalar.activation(
                        out=px[:], in_=s01[:], func=AF.Exp,
                        bias=mask_sb[:, kt:kt + 1], scale=1.0 / math.sqrt(D_KV),
                    )
                    if not NEAR[(kt, qc)]:
                        j0 = kt * P + jg0
                        nc.vector.tensor_tensor(
                            px[:, 0:SC], px[:, 0:SC],
                            u_t[h0][:, j0:j0 + SC], OP.mult
                        )
                        nc.vector.tensor_tensor(
                            px[:, SC:2 * SC], px[:, SC:2 * SC],
                            u_t[h1][:, j0:j0 + SC], OP.mult
                        )
                    nc.tensor.matmul(
                        cx01[0:65, 0:SC], vaug[:, kt, pr, 0:65], px[:, 0:SC],
                        start=(kt == 0), stop=(kt == NKT - 1),
                    )
                    nc.tensor.matmul(
                        cx01[:, SC:2 * SC], vaug[:, kt, pr, 65:VW],
                        px[:, SC:2 * SC],
                        start=(kt == 0), stop=(kt == NKT - 1),
                    )
                    s01 = s01_next

                # proj drain (reversed q for qt)
                if proj is not None:
                    nc.vector.tensor_copy(rev_ap(qt_sb[:, proj, :], jg0), pjq)
                    nc.vector.tensor_copy(
                        kt_sb[:, proj, jg0:jg0 + SC], pjk)

                # ---- normalize + writeback (no DRAM round trip) ----
                # denominators: h0 on psum row 64 (cols 0:512), h1 on row 32
                # (cols 512:1024).  Custom DVE ops (reciprocal_approx_fast)
                # require base-partition-0 operands, so pack both rows into a
                # base-0 [128, 512] staging tile first.
                dnf = stage.tile([P, SC], DT.float32, tag="dnf",
                                 name=f"dnf{pr}_{qc}", bufs=1)
                nc.vector.tensor_copy(dnf[64:65, :], cx01[64:65, 0:SC])
                nc.vector.tensor_copy(dnf[32:33, :], cx01[32:33, SC:2 * SC])
                rb = stage.tile([P, SC], DT.float32, tag="rb",
                                name=f"rb{pr}_{qc}", bufs=1)
                nc.vector.reciprocal_approx_fast(out=rb[:], in_=dnf[:])
                # rb16 is persistent and zeroed once; only rows 64 (rcp h0)
                # and 32 (rcp h1) are ever written, so the K=64 ones-matmuls
                # below pick out exactly those rows.
                nc.vector.tensor_copy(rb16[64:65, :], rb[64:65, :])
                nc.vector.tensor_copy(rb16[32:33, :], rb[32:33, :])
                bc_ps = psum.tile([P, 2 * SC], DT.float32, tag="ps",
                                  name=f"bc{pr}_{qc}")
                nc.tensor.matmul(bc_ps[:, 0:SC], onesb[64:128, :],
                                 rb16[64:128, :], start=True, stop=True)
                nc.tensor.matmul(bc_ps[:, SC:2 * SC], onesb[0:64, :],
                                 rb16[0:64, :], start=True, stop=True)
                bc_sb = stage.tile([P, 2 * SC], DT.bfloat16, tag="bc",
                                   name=f"bcs{pr}_{qc}", bufs=1)
                nc.vector.tensor_copy(bc_sb[:], bc_ps[:])
                nc.vector.tensor_tensor(
                    rev_ap(ctxt[0:64, pr, :], jg0),
                    cx01[0:64, 0:SC], bc_sb[0:64, 0:SC], OP.mult)
                nc.vector.tensor_tensor(
                    rev_ap(ctxt[64:128, pr, :], jg0),
                    cx01[64:128, SC:2 * SC], bc_sb[64:128, SC:2 * SC], OP.mult)

            u_t = load_u(0)
            uraw_next = load_uraw(0, 0)
            for pr in range(NPAIR):
                nxt = pr + 1 if pr + 1 < NPAIR else None
                for qc in range(NQC):
                    uraw_t = uraw_next
                    # prefetch next (pair, qc) raw window
                    if qc + 1 < NQC:
                        uraw_next = load_uraw(pr, qc + 1)
                    elif nxt is not None:
                        uraw_next = load_uraw(nxt, 0)
                    if qc == NQC - 1 and nxt is not None:
                        next_u = load_u(nxt)
                    attn_qc(pr, qc, u_t, uraw_t, nxt)
                if nxt is not None:
                    u_t = next_u

            # ---------- phase D: output projection ----------
            for st in range(NKT):
                oa = psum.tile([P, 2 * SC], DT.float32, tag="ps",
                               name=f"oa{st}")
                ob = psum.tile([P, 2 * SC], DT.float32, tag="ps",
                               name=f"ob{st}")
                o_ps = [oa[:, 0:SC], oa[:, SC:2 * SC],
                        ob[:, 0:SC], ob[:, SC:2 * SC]]
                for m in range(NMT):
                    for nd in range(NQC):
                        nc.tensor.matmul(
                            o_ps[nd], ctxt[:, m, st * P:(st + 1) * P],
                            wo_sb[:, m, nd * SC:(nd + 1) * SC],
                            start=(m == 0), stop=(m == NMT - 1),
                        )
                for half in range(2):
                    o_t = outp.tile([P, 2, SC], DT.float32, tag="out",
                                    name=f"ot{st}_{half}")
                    nc.scalar.copy(o_t[:, 0, :], o_ps[2 * half])
                    nc.vector.tensor_copy(o_t[:, 1, :], o_ps[2 * half + 1])
                    nc.sync.dma_start(
                        out[st * P:(st + 1) * P,
                            half * 2 * SC:(half + 1) * 2 * SC],
                        o_t[:],
                    )

    nc.finalize()
    return nc


_NC_CACHE = None


def _get_nc():
    global _NC_CACHE
    if _NC_CACHE is None:
        _NC_CACHE = _build()
    return _NC_CACHE


def _in_maps(hidden_states, attention_mask, Wq, Wk, Wv, Wo, rel_emb):
    import ml_dtypes
    bf16 = ml_dtypes.bfloat16
    ident = np.eye(P, dtype=np.float32).astype(bf16)
    maps = []
    for c in range(NCORES):
        b, g = c // 4, c % 4
        hlo, hhi = g * HL, (g + 1) * HL
        brel, erel = _bias_tables(
            np.ascontiguousarray(rel_emb[:, hlo:hhi], dtype=np.float32))
        maps.append({
            "xt": np.ascontiguousarray(hidden_states[b].T).astype(bf16),
            "wq": np.ascontiguousarray(Wq[:, hlo * D_KV:hhi * D_KV]).astype(bf16),
            "wk": np.ascontiguousarray(Wk[:, hlo * D_KV:hhi * D_KV]).astype(bf16),
            "wv": np.ascontiguousarray(Wv[:, hlo * D_KV:hhi * D_KV]).astype(bf16),
            "wo": np.ascontiguousarray(Wo[hlo * D_KV:hhi * D_KV, :]).astype(bf16),
            "mask": np.ascontiguousarray(attention_mask[b, 0, 0, :]).astype(np.float32),
            "brel": brel,
            "erel": erel,
            "ident": ident,
        })
    return maps


def kernel(hidden_states, attention_mask, Wq, Wk, Wv, Wo, rel_emb, _trace=False,
           _trace_kwargs=None):
    hidden_states = np.asarray(hidden_states, dtype=np.float32)
    attention_mask = np.asarray(attention_mask, dtype=np.float32)
    Wq = np.asarray(Wq, dtype=np.float32)
    Wk = np.asarray(Wk, dtype=np.float32)
    Wv = np.asarray(Wv, dtype=np.float32)
    Wo = np.asarray(Wo, dtype=np.float32)
    rel_emb = np.asarray(rel_emb, dtype=np.float32)

    nc = _get_nc()
    maps = _in_maps(hidden_states, attention_mask, Wq, Wk, Wv, Wo, rel_emb)
    kw = dict(_trace_kwargs or {})
    res = run_bass_kernel_spmd(nc, maps, core_ids=list(range(NCORES)),
                               trace=_trace, **kw)
    kernel.last_results = res
    outp = np.empty((B, S, D), dtype=np.float32)
    for b in range(B):
        acc = np.asarray(res.results[4 * b]["out"], dtype=np.float32).copy()
        for g in range(1, 4):
            acc += np.asarray(res.results[4 * b + g]["out"], dtype=np.float32)
        outp[b] = acc
    return outp


# revision 13
# speedup vs baseline: 1.2196x; 1.1112x over previous
"""T5-style encoder self-attention (dense_transformer) on 8 Trainium2 NeuronCores.

Problem (full shapes): hidden [2,2048,2048], Wq/Wk/Wv/Wo [2048,2048],
rel_emb [32,32] (bidirectional T5 relative-position bias), mask [2,1,1,2048].

Sharding: data-parallel over batch (2) x tensor-parallel over heads (4 groups
of 8 heads) = 8 cores, Megatron-style. Each core computes a partial output
[2048,2048] for its batch (its 8 heads through its Wo row-slice); the host
sums 4 partials per batch.

Per-core kernel design (bf16 operands, fp32 PSUM accumulation):
  - Both relative-position bias diagonal tables are HOST-computed (they are
    data-dependent only through rel_emb, a [32,32] input): brel = 8*bias
    (log domain, for additive injection) and erel = exp(bias) (for
    multiplicative application), each [8 heads, 4096 diagonals] bf16.
  - Phase B: single pass over x^T computes pair-0 Q^T/K^T and V for ALL
    heads (6 matmuls per x^T tile, PE-bound).  Q^T is stored with s
    REVERSED so the bias becomes a positive-shear Toeplitz.
  - Phase C attention, per (head-pair, q-chunk), k-tile loop pipelined one
    iteration ahead:
      * near-diagonal k-tiles (|k-q| < ~91 somewhere in tile): the bias tile
        is INJECTED into PSUM via an identity matmul (start=True), then the
        two packed QK matmuls accumulate on top; ACT computes
        exp(s/8 + mask + bias) in one shot - no DVE multiply.
      * far k-tiles: bias is exactly constant (bucket 15/31 saturates), but
        rather than bake runtime constants we keep the baseline DVE multiply
        with the erel shear tile (DVE is otherwise idle).
      * next-pair Q/K projection matmuls are interleaved PER k-tile so they
        fill the PE's ACT-wait bubbles (the in-order PE queue can only run
        work that is already emitted ahead of the blocked PV matmul).
  - V augmentation: per pair, even head block = [v(0:64) | ones(64)] (M=65,
    denominator lands on PSUM partition 64), odd head block = 128 wide with
    ones at col 32 and v at cols 64:128 (denominator on partition 32, ctx on
    partitions 64:128).  This keeps every subsequent normalize op
    partition-aligned: reciprocal_approx_fast on the denominator rows, a K=1
    ones-matmul broadcasts the reciprocal across partitions, and a single
    fused DVE tensor_tensor does normalize + un-reverse + bf16 writeback.
    No DRAM round trips.
  - Phase D output projection: for each s-tile, loop m inside nd so
    consecutive matmuls target different PSUM banks; evacuation alternates
    ACT/DVE.
"""

import math
import sys

for _p in ("/opt/trn_rl_repo",):
    if _p not in sys.path:
        sys.path.insert(0, _p)

import numpy as np

import concourse.bass as bass
import concourse.mybir as mybir
import concourse.tile as tile
from concourse import bacc
from concourse.bass_utils import run_bass_kernel_spmd

DT = mybir.dt
AF = mybir.ActivationFunctionType
OP = mybir.AluOpType

# ---- problem constants (hardcoded per contract) ----
B, S, D = 2, 2048, 2048
N_HEADS, D_KV = 32, 64
NUM_BUCKETS, MAX_DISTANCE = 32, 128
NCORES = 8
HL = 8            # heads per core
P = 128
SC = 512          # free-dim chunk
NKT = S // P      # 16 k-tiles
NQC = S // SC     # 4 q-chunks
NDT = D // P      # 16 D-tiles
NMT = (HL * D_KV) // P   # 4 hd m-tiles per core
NPAIR = HL // 2   # 4 head pairs per core
W_U = 3968        # full exp-table shear tile width
W_NEAR = 1152     # near-window raw-table shear tile width
NDIAG = 4096
VW = 193          # vaug per-(kt,pair) width: even block 65 + odd block 128

# near-tile bookkeeping: tile (kt, qc) is "far" iff |k-q| >= 91 everywhere
def _is_near(kt, qc):
    dmin = 128 * kt - 512 * qc - 511
    dmax = 128 * kt + 127 - 512 * qc
    return not (dmin >= 91 or dmax <= -91)

NEAR = {(kt, qc): _is_near(kt, qc) for kt in range(NKT) for qc in range(NQC)}
# raw-table window base per qc (clamped so the shear read stays in bounds)
B0 = [max(0, 1024 * qc - 128) for qc in range(NQC)]
WQC = [min(W_NEAR, NDIAG - 127 - b0) for b0 in B0]


def _rel_bucket_host(d):
    """Exact numpy replica of reference._relative_position_bucket."""
    num_buckets = NUM_BUCKETS // 2          # 16
    max_exact = num_buckets // 2            # 8
    rel = np.asarray(d, dtype=np.int64)
    buckets = (rel > 0).astype(np.int32) * num_buckets
    arel = np.abs(rel)
    is_small = arel < max_exact
    rp_safe = np.maximum(arel, 1).astype(np.float32)
    log_ratio = np.log(rp_safe / np.float32(max_exact)).astype(np.float32)
    scale = np.float32(math.log(MAX_DISTANCE / max_exact))
    rp_large = max_exact + (log_ratio / scale * np.float32(num_buckets - max_exact)).astype(np.int32)
    rp_large = np.minimum(rp_large, num_buckets - 1)
    buckets = buckets + np.where(is_small, arel.astype(np.int32), rp_large)
    return buckets.astype(np.int32)


def _bias_tables(rel_emb_slice):
    """rel_emb_slice: [NUM_BUCKETS, HL] fp32 -> (brel, erel) [HL, NDIAG].
    brel[h, i] = 8 * bias(d = i - 2047); erel[h, i] = exp(bias)."""
    import ml_dtypes
    i = np.arange(NDIAG - 1)
    b = _rel_bucket_host(i - (S - 1))                  # [4095]
    vals = rel_emb_slice[b, :]                         # [4095, HL] fp32
    brel = np.zeros((HL, NDIAG), dtype=np.float32)
    erel = np.zeros((HL, NDIAG), dtype=np.float32)
    brel[:, : NDIAG - 1] = 8.0 * vals.T
    erel[:, : NDIAG - 1] = np.exp(vals.T)
    return (brel.astype(ml_dtypes.bfloat16), erel.astype(ml_dtypes.bfloat16))


def _build():
    nc = bacc.Bacc(None, name="attn_tp")

    xt = nc.declare_dram_parameter("xt", [D, S], DT.bfloat16, isOutput=False)
    # weights arrive HOST-SHUFFLED to [p][kt][h] so per-partition lines are
    # contiguous multi-KB runs (DMA packet rate is the limiter at 1KB lines)
    wq = nc.declare_dram_parameter("wq", [P, NDT * HL * D_KV], DT.bfloat16, isOutput=False)
    wk = nc.declare_dram_parameter("wk", [P, NDT * HL * D_KV], DT.bfloat16, isOutput=False)
    wv = nc.declare_dram_parameter("wv", [P, NDT * HL * D_KV], DT.bfloat16, isOutput=False)
    wo = nc.declare_dram_parameter("wo", [P, NMT * D], DT.bfloat16, isOutput=False)
    mask = nc.declare_dram_parameter("mask", [S], DT.float32, isOutput=False)
    brel = nc.declare_dram_parameter("brel", [HL, NDIAG], DT.bfloat16, isOutput=False)
    erel = nc.declare_dram_parameter("erel", [HL, NDIAG], DT.bfloat16, isOutput=False)
    ident = nc.declare_dram_parameter("ident", [P, P], DT.bfloat16, isOutput=False)
    out = nc.declare_dram_parameter("out", [S, D], DT.float32, isOutput=True)

    with tile.TileContext(nc) as tc:
        with (
            tc.tile_pool(name="res", bufs=1) as res,          # persistent tensors
            tc.tile_pool(name="xtp", bufs=3) as xtp,          # x^T stream tiles
            tc.tile_pool(name="upool", bufs=3) as upool,      # exp-bias shear tiles
            tc.tile_pool(name="urawp", bufs=2) as urawp,      # raw-bias near windows
            tc.tile_pool(name="pexp", bufs=3) as pexpp,       # probs tiles
            tc.tile_pool(name="stage", bufs=2) as stage,      # normalize staging
            tc.tile_pool(name="outp", bufs=3) as outp,        # out staging
            tc.tile_pool(name="psum", bufs=4, space="PSUM") as psum,  # [128,1024] slots
            tc.tile_pool(name="dram", bufs=2, space="DRAM") as dramp,
        ):
            # ---------- constants ----------
            mask_sb = res.tile([P, NKT], DT.float32, tag="mask")
            nc.sync.dma_start(mask_sb[:], mask.ap().rearrange("(kt p) -> p kt", p=P))

            id_sb = res.tile([P, P], DT.bfloat16, tag="ident")
            nc.sync.dma_start(id_sb[:], ident[:])

            # weights (resident, bf16).  wq/wk/wv stream in per-kd chunks on
            # side DMA queues so the xt stream is not blocked at startup; wo
            # loads once on the (idle) scalar queue.
            wq_sb = res.tile([P, NDT, HL * D_KV], DT.bfloat16, tag="wq")
            wk_sb = res.tile([P, NDT, HL * D_KV], DT.bfloat16, tag="wk")
            wv_sb = res.tile([P, NDT, HL * D_KV], DT.bfloat16, tag="wv")
            wo_sb = res.tile([P, NMT, D], DT.bfloat16, tag="wo")

            # persistent activations
            qt_sb = res.tile([P, NMT, S], DT.bfloat16, tag="qt")   # q REVERSED
            kt_sb = res.tile([P, NMT, S], DT.bfloat16, tag="kt")
            vaug = res.tile([P, NKT, NPAIR, VW], DT.bfloat16, tag="vaug")
            ctxt = res.tile([P, NMT, S], DT.bfloat16, tag="ctxt")
            nc.vector.memset(vaug[:], 1.0)

            # ACT exp table warm-up (hide the ~2.7us table load under phase B)
            warm = res.tile([1, 2], DT.float32, tag="warm")
            nc.scalar.activation(out=warm[0:1, 0:1], in_=mask_sb[0:1, 0:1], func=AF.Exp)

            def rev_ap(base, jg0):
                """reversed-q view: base is a [rows, S] AP slice of a res
                tensor; returns [rows, SC] AP walking q backwards so writing
                reversed data lands in natural order."""
                return bass.AP(
                    tensor=base.tensor,
                    offset=base.offset + (S - 1 - jg0),
                    ap=[list(base.ap[0]), [-1, SC]],
                )

            def load_u(pr):
                """full exp-table shear tile [P, 2, W_U] for pair pr."""
                u = upool.tile([P, 2, W_U], DT.bfloat16, tag="u",
                               name=f"u{pr}", bufs=2)
                ap0 = erel.ap()
                for i, hh in enumerate((2 * pr, 2 * pr + 1)):
                    shear = bass.AP(
                        tensor=ap0.tensor,
                        offset=ap0.offset + hh * NDIAG,
                        ap=[[1, P], [1, W_U]],
                    )
                    nc.sync.dma_start(u[:, i, :], shear)
                return u

            def load_uraw(pr, qc):
                """near-window raw-bias shear tile [P, 2, W_NEAR] for (pr, qc)."""
                w = WQC[qc]
                t = urawp.tile([P, 2, W_NEAR], DT.bfloat16, tag="uraw",
                               name=f"uraw{pr}_{qc}")
                ap0 = brel.ap()
                for i, hh in enumerate((2 * pr, 2 * pr + 1)):
                    shear = bass.AP(
                        tensor=ap0.tensor,
                        offset=ap0.offset + hh * NDIAG + B0[qc],
                        ap=[[1, P], [1, w]],
                    )
                    nc.gpsimd.dma_start(t[:, i, 0:w], shear)
                return t

            # ---------- phase B: pair-0 Q/K + V (all heads), single x^T pass ----
            for nq in range(NQC):
                qk_ps = psum.tile([P, 2 * SC], DT.float32, tag="ps",
                                  name=f"qkps0_{nq}")
                q_ps, k_ps = qk_ps[:, 0:SC], qk_ps[:, SC:2 * SC]
                v01 = psum.tile([P, 2 * SC], DT.float32, tag="ps", name=f"v01_{nq}")
                v23 = psum.tile([P, 2 * SC], DT.float32, tag="ps", name=f"v23_{nq}")
                v_ps = [v01[:, 0:SC], v01[:, SC:2 * SC],
                        v23[:, 0:SC], v23[:, SC:2 * SC]]
                for kd in range(NDT):
                    if nq == 0 and kd % 4 == 0:
                        g = kd // 4
                        c0, c1 = g * 4 * HL * D_KV, (g + 1) * 4 * HL * D_KV
                        nc.gpsimd.dma_start(wq_sb[:, g * 4:(g + 1) * 4, :],
                                            wq[:, c0:c1])
                        nc.gpsimd.dma_start(wk_sb[:, g * 4:(g + 1) * 4, :],
                                            wk[:, c0:c1])
                        nc.gpsimd.dma_start(wv_sb[:, g * 4:(g + 1) * 4, :],
                                            wv[:, c0:c1])
                    xt_t = xtp.tile([P, SC], DT.bfloat16, tag="xt",
                                    name=f"xb{nq}_{kd}")
                    eng = nc.sync if kd % 2 == 0 else nc.scalar
                    eng.dma_start(
                        xt_t[:], xt[kd * P:(kd + 1) * P, nq * SC:(nq + 1) * SC]
                    )
                    nc.tensor.matmul(
                        q_ps, wq_sb[:, kd, 0:P], xt_t[:],
                        start=(kd == 0), stop=(kd == NDT - 1),
                    )
                    nc.tensor.matmul(
                        k_ps, wk_sb[:, kd, 0:P], xt_t[:],
                        start=(kd == 0), stop=(kd == NDT - 1),
                    )
                    for st in range(4):
                        nc.tensor.matmul(
                            v_ps[st], xt_t[:, st * P:(st + 1) * P],
                            wv_sb[:, kd, :],
                            start=(kd == 0), stop=(kd == NDT - 1),
                        )
                if nq == 0:
                    nc.gpsimd.dma_start(
                        wo_sb.rearrange("p a b -> p (a b)"), wo[:])
                # drain: q (reversed) + k casts, V -> vaug blocks
                nc.vector.tensor_copy(rev_ap(qt_sb[:, 0, :], nq * SC), q_ps)
                nc.vector.tensor_copy(kt_sb[:, 0, nq * SC:(nq + 1) * SC], k_ps)
                for st in range(4):
                    ktg = nq * 4 + st
                    vsrc = v_ps[st].rearrange("p (pr par d) -> p pr par d",
                                              par=2, d=D_KV)
                    nc.vector.tensor_copy(vaug[:, ktg, :, 0:D_KV],
                                          vsrc[:, :, 0, :])
                    nc.vector.tensor_copy(vaug[:, ktg, :, 129:193],
                                          vsrc[:, :, 1, :])

            # ---------- phase C: attention, proj of pair pr+1 interleaved ----
            def attn_qc(pr, qc, u_t, uraw_t, proj, pending):
                """attention for head pair pr, reversed-q chunk qc.
                proj: None or pr+1 (emit that pair's Q/K proj, 1 kd per kt).
                Emission order per kt puts all independent PE work BEFORE the
                dependent PV matmuls so the in-order PE queue can fill
                ACT-wait bubbles."""
                h0, h1 = 2 * pr, 2 * pr + 1
                jg0 = qc * SC
                cx01 = psum.tile([P, 2 * SC], DT.float32, tag="ps",
                                 name=f"cx{pr}_{qc}")
                if proj is not None:
                    pj_ps = psum.tile([P, 2 * SC], DT.float32, tag="ps",
                                      name=f"pjps{proj}_{qc}")
                    pjq, pjk = pj_ps[:, 0:SC], pj_ps[:, SC:2 * SC]

                def emit_s(kt):
                    s01 = psum.tile([P, 2 * SC], DT.float32, tag="ps",
                                    name=f"s{pr}_{qc}_{kt}")
                    near = NEAR[(kt, qc)]
                    j0 = kt * P + jg0
                    if near:
                        a = j0 - B0[qc]
                        nc.tensor.matmul(
                            s01[:, 0:SC], id_sb[:], uraw_t[:, 0, a:a + SC],
                            start=True, stop=False,
                        )
                        nc.tensor.matmul(
                            s01[:, SC:2 * SC], id_sb[:], uraw_t[:, 1, a:a + SC],
                            start=True, stop=False,
                        )
                    nc.tensor.matmul(
                        s01[:, 0:SC], kt_sb[0:64, pr, kt * P:(kt + 1) * P],
                        qt_sb[0:64, pr, jg0:jg0 + SC],
                        start=not near, stop=True, tile_position=(0, 0),
                    )
                    nc.tensor.matmul(
                        s01[:, SC:2 * SC], kt_sb[64:128, pr, kt * P:(kt + 1) * P],
                        qt_sb[64:128, pr, jg0:jg0 + SC],
                        start=not near, stop=True, tile_position=(64, 0),
                    )
                    return s01

                def emit_proj(kd):
                    xt_t = xtp.tile([P, SC], DT.bfloat16, tag="xt",
                                    name=f"xp{proj}_{qc}_{kd}")
                    eng = nc.sync if kd % 2 == 0 else nc.gpsimd
                    eng.dma_start(
                        xt_t[:], xt[kd * P:(kd + 1) * P, jg0:jg0 + SC]
                    )
                    nc.tensor.matmul(
                        pjq, wq_sb[:, kd, proj * P:(proj + 1) * P], xt_t[:],
                        start=(kd == 0), stop=(kd == NDT - 1),
                    )
                    nc.tensor.matmul(
                        pjk, wk_sb[:, kd, proj * P:(proj + 1) * P], xt_t[:],
                        start=(kd == 0), stop=(kd == NDT - 1),
                    )

                # 2-deep software pipeline: s(kt+2) is emitted before PV(kt)
                # so the in-order PE queue keeps a backlog (hides LDWEIGHTS
                # and cross-engine semaphore latency).  pending() emits the
                # PREVIOUS qc's deferred normalize chain (DVE+DMA only).
                sq = [emit_s(0), emit_s(1)]
                for kt in range(NKT):
                    if kt + 2 < NKT:
                        sq.append(emit_s(kt + 2))
                    if proj is not None:
                        emit_proj(kt)
                    if kt == 2 and pending is not None:
                        pending()
                    s01 = sq[kt]
                    px = pexpp.tile([P, 2 * SC], DT.bfloat16, tag="pexp",
                                    name=f"px{pr}_{qc}_{kt}")
                    nc.scalar.activation(
                        out=px[:], in_=s01[:], func=AF.Exp,
                        bias=mask_sb[:, kt:kt + 1], scale=1.0 / math.sqrt(D_KV),
                    )
                    if not NEAR[(kt, qc)]:
                        j0 = kt * P + jg0
                        nc.vector.tensor_tensor(
                            px.rearrange("p (h j) -> p h j", h=2),
                            px.rearrange("p (h j) -> p h j", h=2),
                            u_t[:, :, j0:j0 + SC], OP.mult
                        )
                    nc.tensor.matmul(
                        cx01[0:65, 0:SC], vaug[:, kt, pr, 0:65], px[:, 0:SC],
                        start=(kt == 0), stop=(kt == NKT - 1),
                    )
                    nc.tensor.matmul(
                        cx01[:, SC:2 * SC], vaug[:, kt, pr, 65:VW],
                        px[:, SC:2 * SC],
                        start=(kt == 0), stop=(kt == NKT - 1),
                    )

                # proj drain (reversed q for qt)
                if proj is not None:
                    nc.vector.tensor_copy(rev_ap(qt_sb[:, proj, :], jg0), pjq)
                    nc.vector.tensor_copy(
                        kt_sb[:, proj, jg0:jg0 + SC], pjk)

                # ---- evacuate cx to SBUF (frees the PSUM slot), then the
                # rest of normalize+writeback is DEFERRED into the next qc
                # (DVE + DMA only; the PE never touches it) ----
                cxs = stage.tile([P, 2 * SC], DT.float32, tag="cxs",
                                 name=f"cxs{pr}_{qc}", bufs=1)
                nc.vector.tensor_copy(cxs[:], cx01[:])

                def normalize():
                    # denominators: h0 on row 64 (cols 0:512), h1 on row 32
                    # (cols 512:1024).  Custom DVE ops need base-partition-0
                    # operands, so pack both rows into a base-0 tile first.
                    dnf = stage.tile([P, SC], DT.float32, tag="dnf",
                                     name=f"dnf{pr}_{qc}", bufs=1)
                    nc.vector.tensor_copy(dnf[64:65, :], cxs[64:65, 0:SC])
                    nc.vector.tensor_copy(dnf[32:33, :], cxs[32:33, SC:2 * SC])
                    rb = stage.tile([P, SC], DT.float32, tag="rb",
                                    name=f"rb{pr}_{qc}", bufs=1)
                    nc.vector.reciprocal_approx_fast(out=rb[:], in_=dnf[:])
                    # broadcast across partitions: bounce the two reciprocal
                    # rows through DRAM, then stride-0 DMA reads replicate
                    # them to 64 partitions each (all off the engine queues).
                    bnc = dramp.tile([2, SC], DT.float32, tag="bnc",
                                     name=f"bnc{pr}_{qc}")
                    nc.gpsimd.dma_start(bnc[0:1, :], rb[64:65, :])
                    nc.gpsimd.dma_start(bnc[1:2, :], rb[32:33, :])
                    bc_sb = stage.tile([P, SC], DT.float32, tag="bc",
                                       name=f"bcs{pr}_{qc}", bufs=1)
                    src0 = bass.AP(tensor=bnc.tensor, offset=bnc.offset,
                                   ap=[[0, 64], [1, SC]])
                    src1 = bass.AP(tensor=bnc.tensor, offset=bnc.offset + SC,
                                   ap=[[0, 64], [1, SC]])
                    nc.gpsimd.dma_start(bc_sb[0:64, :], src0)
                    nc.gpsimd.dma_start(bc_sb[64:128, :], src1)
                    nc.vector.tensor_tensor(
                        rev_ap(ctxt[0:64, pr, :], jg0),
                        cxs[0:64, 0:SC], bc_sb[0:64, :], OP.mult)
                    nc.vector.tensor_tensor(
                        rev_ap(ctxt[64:128, pr, :], jg0),
                        cxs[64:128, SC:2 * SC], bc_sb[64:128, :], OP.mult)
                return normalize

            u_t = load_u(0)
            uraw_next = load_uraw(0, 0)
            pending = None
            for pr in range(NPAIR):
                nxt = pr + 1 if pr + 1 < NPAIR else None
                for qc in range(NQC):
                    uraw_t = uraw_next
                    # prefetch next (pair, qc) raw window
                    if qc + 1 < NQC:
                        uraw_next = load_uraw(pr, qc + 1)
                    elif nxt is not None:
                        uraw_next = load_uraw(nxt, 0)
                    if qc == NQC - 1 and nxt is not None:
                        next_u = load_u(nxt)
                    pending = attn_qc(pr, qc, u_t, uraw_t, nxt, pending)
                if nxt is not None:
                    u_t = next_u
            pending()

            # ---------- phase D: output projection ----------
            for st in range(NKT):
                oa = psum.tile([P, 2 * SC], DT.float32, tag="ps",
                               name=f"oa{st}")
                ob = psum.tile([P, 2 * SC], DT.float32, tag="ps",
                               name=f"ob{st}")
                o_ps = [oa[:, 0:SC], oa[:, SC:2 * SC],
                        ob[:, 0:SC], ob[:, SC:2 * SC]]
                for m in range(NMT):
                    for nd in range(NQC):
                        nc.tensor.matmul(
                            o_ps[nd], ctxt[:, m, st * P:(st + 1) * P],
                            wo_sb[:, m, nd * SC:(nd + 1) * SC],
                            start=(m == 0), stop=(m == NMT - 1),
                        )
                for half in range(2):
                    o_t = outp.tile([P, 2, SC], DT.float32, tag="out",
                                    name=f"ot{st}_{half}")
                    nc.scalar.copy(o_t[:, 0, :], o_ps[2 * half])
                    nc.vector.tensor_copy(o_t[:, 1, :], o_ps[2 * half + 1])
                    nc.sync.dma_start(
                        out[st * P:(st + 1) * P,
                            half * 2 * SC:(half + 1) * 2 * SC],
                        o_t[:],
                    )

    nc.finalize()
    return nc


_NC_CACHE = None


def _get_nc():
    global _NC_CACHE
    if _NC_CACHE is None:
        _NC_CACHE = _build()
    return _NC_CACHE


def _in_maps(hidden_states, attention_mask, Wq, Wk, Wv, Wo, rel_emb):
    import ml_dtypes
    bf16 = ml_dtypes.bfloat16
    ident = np.eye(P, dtype=np.float32).astype(bf16)
    maps = []
    for c in range(NCORES):
        b, g = c // 4, c % 4
        hlo, hhi = g * HL, (g + 1) * HL
        brel, erel = _bias_tables(
            np.ascontiguousarray(rel_emb[:, hlo:hhi], dtype=np.float32))
        def shuf(w):  # [NDT*P, C] -> [P, NDT*C] partition-contiguous
            c = w.shape[1]
            return np.ascontiguousarray(
                w.reshape(-1, P, c).transpose(1, 0, 2).reshape(P, -1))
        maps.append({
            "xt": np.ascontiguousarray(hidden_states[b].T).astype(bf16),
            "wq": shuf(Wq[:, hlo * D_KV:hhi * D_KV]).astype(bf16),
            "wk": shuf(Wk[:, hlo * D_KV:hhi * D_KV]).astype(bf16),
            "wv": shuf(Wv[:, hlo * D_KV:hhi * D_KV]).astype(bf16),
            "wo": shuf(Wo[hlo * D_KV:hhi * D_KV, :]).astype(bf16),
            "mask": np.ascontiguousarray(attention_mask[b, 0, 0, :]).astype(np.float32),
            "brel": brel,
            "erel": erel,
            "ident": ident,
        })
    return maps


def kernel(hidden_states, attention_mask, Wq, Wk, Wv, Wo, rel_emb, _trace=False,
           _trace_kwargs=None):
    hidden_states = np.asarray(hidden_states, dtype=np.float32)
    attention_mask = np.asarray(attention_mask, dtype=np.float32)
    Wq = np.asarray(Wq, dtype=np.float32)
    Wk = np.asarray(Wk, dtype=np.float32)
    Wv = np.asarray(Wv, dtype=np.float32)
    Wo = np.asarray(Wo, dtype=np.float32)
    rel_emb = np.asarray(rel_emb, dtype=np.float32)

    nc = _get_nc()
    maps = _in_maps(hidden_states, attention_mask, Wq, Wk, Wv, Wo, rel_emb)
    kw = dict(_trace_kwargs or {})
    res = run_bass_kernel_spmd(nc, maps, core_ids=list(range(NCORES)),
                               trace=_trace, **kw)
    kernel.last_results = res
    outp = np.empty((B, S, D), dtype=np.float32)
    for b in range(B):
        acc = np.asarray(res.results[4 * b]["out"], dtype=np.float32).copy()
        for g in range(1, 4):
            acc += np.asarray(res.results[4 * b + g]["out"], dtype=np.float32)
        outp[b] = acc
    return outp
